# revision 17
# baseline (speedup 1.0000x reference)
"""BiMambaBlock Trainium2 Bass kernel (v2).

Sharding: 8 cores = (batch b in {0,1}) x (branch r in {fwd,bwd}) x
(d_inner half h in {0,1}).  Each core runs the same SPMD program on its
shard.

v2 restructure vs v1 (HW-calibrated: DVE scan ~2cyc/el ~4.4us, Pool TT
~3.8us AND fully serializing with DVE scans on the shared SBUF port,
DVE TT bf16 ~1.1us, ACT ~1.8us per [128,2048] op):
  - x is normalized in place up front (PE ones-matmul stats), so the
    in_proj PSUM results need only a plain copy/silu instead of the v1
    per-block mean/rstd fixups.
  - causal depthwise conv = 4 shifted diag-matmuls accumulated in PSUM
    (PE), bias+silu fused into the ACT copy-out.
  - selective scan in D-pairs (two full-width PSUM accumulators): per
    (D,n): ACT exp -> DVE B-mul -> DVE hardware scan -> DVE C-mul -> PE
    identity-matmul accumulation into PSUM (replaces per-state adds),
    seeded by a diag(D)-matmul of u.  All elementwise muls stay on DVE:
    Pool is measured to serialize with DVE scans, so offloading to it
    earns nothing, and its per-op cost is 3.4x DVE's.
  - B/C broadcast DMAs are issued alternately from the SP and ACT
    queues (issue serialization on one queue cost ~290us).
  - dt = softplus via ln(1+exp(w)): Exp and Ln share one ACT table set,
    so the 4-op DVE series becomes 2 ACT ops + 1 DVE add.
  - gating drains PSUM through an ACT copy so the DVE gate-mul runs at
    2x on bf16 instead of 1x from PSUM.
  - fused out_proj@final_proj matmul, scoped PSUM pools per phase.
Host side only shards/flips inputs, folds weights, and sums the partial
outputs (row-parallel gather) plus residual.
"""

import os
import sys

for _p in ("/opt/trn_rl_repo", "/root/.axon_site/_ro/trn_rl_repo"):
    if os.path.isdir(_p) and _p not in sys.path:
        sys.path.insert(0, _p)
        break

import numpy as np
import ml_dtypes

import concourse.bass as bass
import concourse.mybir as mybir
import concourse.tile as tile
from concourse import bacc

BF16 = ml_dtypes.bfloat16
F32 = mybir.dt.float32
BF = mybir.dt.bfloat16

D_MODEL = 1024
D_INNER = 2048
D_STATE = 16
D_CONV = 4
DT_RANK = 64
BATCH, SEQ = 2, 2048
DL = 1024          # local d_inner half per core
NBLK = DL // 128   # 8 d-blocks of 128
NTC = SEQ // 512   # 4 time chunks of 512 for matmuls
NMT = SEQ // 128   # 16 time tiles of 128 for output matmul

MULT = mybir.AluOpType.mult
ADD = mybir.AluOpType.add
SUB = mybir.AluOpType.subtract
AF = mybir.ActivationFunctionType

# engine-assignment knob: of the 16 states, how many C-muls go to Pool
N_CMUL_POOL = 0


def _build_body(nc, tc, tensors):
    (xT, w_inT, xproj_wT, dt_wT, w_foldT, conv_diag, dp_diag, ident_p,
     conv_b_c, silu_zb_c, dt_b_c, a_cols, y_part,
     cc_in, cc_out, bc_dram, gate_dram, bench) = tensors

    with (
        tc.tile_pool(name="pc", bufs=1) as pc,            # constants
        tc.tile_pool(name="px", bufs=10) as px,           # x/mu/rstd -> yg
        tc.tile_pool(name="pu", bufs=8) as pu,            # u tiles
        tc.tile_pool(name="pxr", bufs=8) as pxr,          # xr (padded) -> dt
        tc.tile_pool(name="pgs", bufs=2) as pgs,          # gate stream-in
        tc.tile_pool(name="pwin", bufs=8) as pwin,        # w_inT -> scan transients
        tc.tile_pool(name="pdtu", bufs=2) as pdtu,        # dtu per D
        tc.tile_pool(name="pbc", bufs=5) as pbc,          # brep/crep
        tc.tile_pool(name="pwf", bufs=8) as pwf,          # w_fold tiles
        tc.tile_pool(name="psmall", bufs=2) as psmall,    # [128,512] transients
        tc.tile_pool(name="pdbc", bufs=1) as pdbc,        # dbc
    ):
        # ---- constants ----
        ones_m = pc.tile([128, 128], BF, tag="ones", name="ones")
        nc.vector.memset(ones_m, 1.0 / D_MODEL)
        ident = pc.tile([128, 128], BF, tag="ident", name="ident")
        nc.sync.dma_start(out=ident, in_=ident_p[:])
        dpd = pc.tile([128, NBLK * 128], BF, tag="dpd", name="dpd")
        nc.sync.dma_start(out=dpd, in_=dp_diag[:])
        cwd = pc.tile([128, NBLK * D_CONV * 128], BF, tag="cwd", name="cwd")
        nc.sync.dma_start(out=cwd, in_=conv_diag[:])
        convb = pc.tile([128, NBLK], F32, tag="convb", name="convb")
        nc.sync.dma_start(out=convb, in_=conv_b_c[:])
        szb = pc.tile([128, NBLK], F32, tag="szb", name="szb")
        nc.sync.dma_start(out=szb, in_=silu_zb_c[:])
        dtb = pc.tile([128, NBLK], F32, tag="dtb", name="dtb")
        nc.sync.dma_start(out=dtb, in_=dt_b_c[:])
        acol = pc.tile([128, NBLK * D_STATE], F32, tag="acol", name="acol")
        nc.sync.dma_start(out=acol, in_=a_cols[:])
        epsb = pc.tile([128, 1], F32, tag="epsb", name="epsb")
        nc.vector.memset(epsb, 1e-5)
        xpw = pc.tile([128, NBLK * 96], BF, tag="xpw", name="xpw")
        for D in range(NBLK):
            nc.sync.dma_start(out=xpw[:, D * 96:(D + 1) * 96],
                              in_=xproj_wT[D * 128:(D + 1) * 128, :])
        dtw = pc.tile([DT_RANK, DL], BF, tag="dtw", name="dtw")
        nc.sync.dma_start(out=dtw, in_=dt_wT[:])

        # ================= front: stats/in_proj/conv/xproj/dt =================
        with tc.tile_pool(name="psf", bufs=4, space="PSUM") as ps:
            # ---- phase 1: load x, LN stats via PE ones-matmul ----
            xbf = []
            for D in range(NBLK):
                t = px.tile([128, SEQ], BF, tag="big", name="xbf")
                nc.sync.dma_start(out=t, in_=xT[D * 128:(D + 1) * 128, :])
                xbf.append(t)

            mu_f = px.tile([128, SEQ], BF, tag="big", name="mu")
            rstd_f = px.tile([128, SEQ], BF, tag="big", name="rstd")
            for c in range(NTC):
                sl = bass.ts(c, 512)
                mu_ps = ps.tile([128, 512], F32, tag="ps", name="mups")
                ex2_ps = ps.tile([128, 512], F32, tag="ps", name="exps")
                for D in range(NBLK):
                    xsq = psmall.tile([128, 512], BF, tag="sm", name="xsq")
                    nc.vector.tensor_mul(xsq, xbf[D][:, sl], xbf[D][:, sl])
                    nc.tensor.matmul(mu_ps[:], ones_m[:], xbf[D][:, sl],
                                     start=(D == 0), stop=(D == NBLK - 1))
                    nc.tensor.matmul(ex2_ps[:], ones_m[:], xsq[:],
                                     start=(D == 0), stop=(D == NBLK - 1))
                nc.scalar.activation(mu_f[:, sl], mu_ps[:], AF.Copy)
                v = psmall.tile([128, 512], F32, tag="sm", name="vv")
                nc.vector.tensor_mul(v, mu_f[:, sl], mu_f[:, sl])
                nc.vector.tensor_sub(v, ex2_ps[:], v)
                nc.scalar.activation(v, v, AF.Sqrt, bias=epsb[:, 0:1])
                nc.vector.reciprocal(v, v)
                nc.vector.tensor_copy(rstd_f[:, sl], v)

            # ---- phase 1.5: normalize x in place (DVE; Pool serializes
            # with DVE on the shared SBUF port, so it earns nothing) ----
            for D in range(NBLK):
                nc.vector.tensor_sub(xbf[D], xbf[D], mu_f)
                nc.vector.tensor_mul(xbf[D], xbf[D], rstd_f)

            # ---- phase 2+3: in_proj (c-outer) + conv + x_proj partials ----
            winT = []
            for D in range(NBLK):
                t = pwin.tile([128, 2 * DL], BF, tag="w", name="w")
                nc.sync.dma_start(out=t, in_=w_inT[D * 128:(D + 1) * 128, :])
                winT.append(t)
            xr = []
            for D in range(NBLK):
                t = pxr.tile([128, 3 + SEQ], BF, tag="xr", name="xr")
                nc.vector.memset(t[:, 0:3], 0.0)
                xr.append(t)
            u = []
            for D in range(NBLK):
                u.append(pu.tile([128, SEQ], BF, tag="u", name="u"))

            for c in range(NTC):
                sl = bass.ts(c, 512)
                for m in range(16):
                    pxz = ps.tile([128, 512], F32, tag="ps", name="pxz")
                    for D in range(NBLK):
                        nc.tensor.matmul(pxz[:], winT[D][:, bass.ts(m, 128)],
                                         xbf[D][:, sl],
                                         start=(D == 0), stop=(D == NBLK - 1))
                    if m < NBLK:
                        nc.scalar.activation(
                            xr[m][:, 3 + c * 512:3 + (c + 1) * 512],
                            pxz[:], AF.Copy)
                    else:
                        gst = psmall.tile([128, 512], BF, tag="sm", name="gst")
                        nc.scalar.activation(gst, pxz[:], AF.Silu,
                                             bias=szb[:, m - NBLK:m - NBLK + 1])
                        nc.sync.dma_start(
                            out=gate_dram[(m - NBLK) * 128:(m - NBLK + 1) * 128, sl],
                            in_=gst)
                # conv for this chunk (uses xr chunk c of all D)
                for D in range(NBLK):
                    pcv = ps.tile([128, 512], F32, tag="ps", name="pcv")
                    for k in range(D_CONV):
                        nc.tensor.matmul(
                            pcv[:],
                            cwd[:, (D * D_CONV + k) * 128:(D * D_CONV + k + 1) * 128],
                            xr[D][:, k + c * 512:k + c * 512 + 512],
                            start=(k == 0), stop=(k == D_CONV - 1))
                    nc.scalar.activation(u[D][:, sl], pcv[:], AF.Silu,
                                         bias=convb[:, D:D + 1])
                # x_proj partial for this chunk
                pdbc_ps = ps.tile([128, 512], F32, tag="ps", name="pdbc")
                for D in range(NBLK):
                    nc.tensor.matmul(pdbc_ps[0:96, :], xpw[:, D * 96:(D + 1) * 96],
                                     u[D][:, sl],
                                     start=(D == 0), stop=(D == NBLK - 1))
                dst = psmall.tile([96, 512], BF, tag="sm", name="dbcst")
                nc.scalar.activation(dst, pdbc_ps[0:96, :], AF.Copy)
                nc.sync.dma_start(out=cc_in[0:96, sl], in_=dst)

            # ---- phase 4: pair AllReduce over the d_inner halves ----
            if bench:
                nc.sync.dma_start(out=cc_out[:], in_=cc_in[:])
            else:
                nc.gpsimd.collective_compute(
                    "AllReduce", ADD,
                    replica_groups=[[0, 1], [2, 3], [4, 5], [6, 7]],
                    ins=[cc_in[:]], outs=[cc_out[:]])
            dbc = pdbc.tile([96, SEQ], BF, tag="dbc", name="dbc")
            nc.sync.dma_start(out=dbc, in_=cc_out[:])
            nc.sync.dma_start(out=bc_dram[:], in_=dbc[DT_RANK:96, :])

            # ---- phase 5: dt = softplus series ----
            dt = []
            for D in range(NBLK):
                dtt = pxr.tile([128, 3 + SEQ], BF, tag="xr", name="dt")
                for c in range(NTC):
                    pdt = ps.tile([128, 512], F32, tag="ps", name="pdt")
                    nc.tensor.matmul(pdt[:], dtw[:, bass.ts(D, 128)],
                                     dbc[0:DT_RANK, bass.ts(c, 512)],
                                     start=True, stop=True)
                    # softplus(x) = ln(1 + e^x); Exp and Ln share one ACT
                    # table set, so this costs 2 ACT ops + 1 cheap DVE add
                    # instead of the 4-DVE-op series
                    ex = psmall.tile([128, 512], F32, tag="sm", name="spx")
                    nc.scalar.activation(ex, pdt[:], AF.Exp, bias=dtb[:, D:D + 1])
                    nc.vector.tensor_scalar(ex, ex, 1.0, None, op0=ADD)
                    nc.scalar.activation(dtt[:, 3 + c * 512:3 + (c + 1) * 512],
                                         ex, AF.Ln)
                dt.append(dtt)

        # ============ scan: D-pairs, 2 full-width PSUM accumulators ============
        yg = [None] * NBLK
        with tc.tile_pool(name="psa", bufs=2, space="PSUM") as psacc:
            dma_engines = [nc.sync, nc.scalar]
            for pair in range(NBLK // 2):
                Ds = (2 * pair, 2 * pair + 1)
                acc = {}
                dtu = {}
                for D in Ds:
                    dtu[D] = pdtu.tile([128, SEQ], BF, tag="dtu", name="dtu")
                    nc.vector.tensor_mul(dtu[D], dt[D][:, 3:3 + SEQ], u[D])
                    acc[D] = psacc.tile([128, SEQ], F32, tag="acc", name="acc")
                    for c in range(NTC):
                        nc.tensor.matmul(
                            acc[D][:, bass.ts(c, 512)],
                            dpd[:, D * 128:(D + 1) * 128],
                            u[D][:, bass.ts(c, 512)],
                            start=True, stop=False)
                for n in range(D_STATE):
                    eng = dma_engines[n % len(dma_engines)]
                    brep = pbc.tile([128, SEQ], BF, tag="bc", name="brep")
                    src = bc_dram[n:n + 1, :]
                    eng.dma_start(out=brep, in_=bass.AP(
                        tensor=src.tensor, offset=src.offset,
                        ap=[[0, 128]] + list(src.ap[1:])))
                    crep = pbc.tile([128, SEQ], BF, tag="bc", name="crep")
                    src = bc_dram[D_STATE + n:D_STATE + n + 1, :]
                    eng.dma_start(out=crep, in_=bass.AP(
                        tensor=src.tensor, offset=src.offset,
                        ap=[[0, 128]] + list(src.ap[1:])))
                    for D in Ds:
                        av = pwin.tile([128, SEQ], BF, tag="w", name="av")
                        nc.scalar.activation(
                            av, dt[D][:, 3:3 + SEQ], AF.Exp,
                            scale=acol[:, D * D_STATE + n:D * D_STATE + n + 1])
                        bv = pwin.tile([128, SEQ], BF, tag="w", name="bv")
                        nc.vector.tensor_mul(bv, dtu[D], brep)
                        nc.vector.tensor_tensor_scan(av, av, bv, 0.0,
                                                     op0=MULT, op1=ADD)
                        if n < N_CMUL_POOL:
                            nc.gpsimd.tensor_mul(bv, av, crep)
                        else:
                            nc.vector.tensor_mul(bv, av, crep)
                        for c in range(NTC):
                            nc.tensor.matmul(
                                acc[D][:, bass.ts(c, 512)], ident[:],
                                bv[:, bass.ts(c, 512)],
                                start=False, stop=(n == D_STATE - 1))
                # gating: yg = acc * silu(z), direct from PSUM on DVE
                for D in Ds:
                    ygt = px.tile([128, SEQ], BF, tag="big", name="yg")
                    for h in range(2):
                        g = pgs.tile([128, 1024], BF, tag="gs", name="g")
                        nc.sync.dma_start(
                            out=g,
                            in_=gate_dram[D * 128:(D + 1) * 128,
                                          h * 1024:(h + 1) * 1024])
                        # drain PSUM via ACT so the DVE mul runs 2x on bf16
                        nc.scalar.activation(ygt[:, bass.ts(h, 1024)],
                                             acc[D][:, bass.ts(h, 1024)],
                                             AF.Copy)
                        nc.vector.tensor_mul(ygt[:, bass.ts(h, 1024)],
                                             ygt[:, bass.ts(h, 1024)], g)
                    yg[D] = ygt

        # ================= tail: fused out_proj @ proj =================
        with tc.tile_pool(name="pst", bufs=4, space="PSUM") as pso:
            wf = []
            for D in range(NBLK):
                t = pwf.tile([128, D_MODEL], BF, tag="wf", name="wf")
                nc.sync.dma_start(out=t, in_=w_foldT[D * 128:(D + 1) * 128, :])
                wf.append(t)
            for m in range(NMT):
                for oc in range(2):
                    po = pso.tile([128, 512], F32, tag="ps", name="po")
                    for D in range(NBLK):
                        nc.tensor.matmul(po[:], yg[D][:, bass.ts(m, 128)],
                                         wf[D][:, bass.ts(oc, 512)],
                                         start=(D == 0), stop=(D == NBLK - 1))
                    ot = psmall.tile([128, 512], F32, tag="sm", name="ot")
                    k = m * 2 + oc
                    if k % 2 == 0:
                        nc.scalar.activation(ot, po[:], AF.Copy)
                    else:
                        nc.vector.tensor_copy(ot, po[:])
                    nc.sync.dma_start(
                        out=y_part[m * 128:(m + 1) * 128, bass.ts(oc, 512)],
                        in_=ot)


def _build_program(bench=False, reps=1):
    nc = bacc.Bacc("TRN2", target_bir_lowering=False, debug=False, num_devices=8)

    xT = nc.declare_dram_parameter("xT", [D_MODEL, SEQ], BF, isOutput=False)
    w_inT = nc.declare_dram_parameter("w_inT", [D_MODEL, 2 * DL], BF, isOutput=False)
    xproj_wT = nc.declare_dram_parameter("xproj_wT", [DL, 96], BF, isOutput=False)
    dt_wT = nc.declare_dram_parameter("dt_wT", [DT_RANK, DL], BF, isOutput=False)
    w_foldT = nc.declare_dram_parameter("w_foldT", [DL, D_MODEL], BF, isOutput=False)
    conv_diag = nc.declare_dram_parameter("conv_diag", [128, NBLK * D_CONV * 128], BF, isOutput=False)
    dp_diag = nc.declare_dram_parameter("dp_diag", [128, NBLK * 128], BF, isOutput=False)
    ident_p = nc.declare_dram_parameter("ident_p", [128, 128], BF, isOutput=False)
    conv_b_c = nc.declare_dram_parameter("conv_b_c", [128, NBLK], F32, isOutput=False)
    silu_zb_c = nc.declare_dram_parameter("silu_zb_c", [128, NBLK], F32, isOutput=False)
    dt_b_c = nc.declare_dram_parameter("dt_b_c", [128, NBLK], F32, isOutput=False)
    a_cols = nc.declare_dram_parameter("a_cols", [128, NBLK * D_STATE], F32, isOutput=False)

    y_part = nc.declare_dram_parameter("y_part", [SEQ, D_MODEL], F32, isOutput=True)

    cc_in = nc.dram_tensor("cc_in", [96, SEQ], BF)
    cc_out = nc.dram_tensor("cc_out", [96, SEQ], BF)
    bc_dram = nc.dram_tensor("bc_dram", [2 * D_STATE, SEQ], BF)
    gate_dram = nc.dram_tensor("gate_dram", [DL, SEQ], BF)

    tensors = (xT, w_inT, xproj_wT, dt_wT, w_foldT, conv_diag, dp_diag, ident_p,
               conv_b_c, silu_zb_c, dt_b_c, a_cols, y_part,
               cc_in, cc_out, bc_dram, gate_dram, bench)
    for _rep in range(reps):
        with tile.TileContext(nc) as tc:
            _build_body(nc, tc, tensors)
    nc.compile()
    return nc


_CACHE = {}


def _make_runner(nc):
    import jax
    from jax.sharding import Mesh, PartitionSpec, NamedSharding
    from jax.experimental.shard_map import shard_map
    from concourse import bass2jax

    bass2jax.install_neuronx_cc_hook()
    partition_name = nc.partition_id_tensor.name if nc.partition_id_tensor else None
    in_names, out_names, out_avals, zero_outs = [], [], [], []
    for alloc in nc.m.functions[0].allocations:
        if not isinstance(alloc, mybir.MemoryLocationSet):
            continue
        name = alloc.memorylocations[0].name
        if alloc.kind == "ExternalInput":
            if name != partition_name:
                in_names.append(name)
        elif alloc.kind == "ExternalOutput":
            out_names.append(name)
            shape = tuple(alloc.tensor_shape)
            dtype = mybir.dt.np(alloc.dtype)
            out_avals.append(jax.core.ShapedArray(shape, dtype))
            zero_outs.append(np.zeros(shape, dtype))
    n_params = len(in_names)
    all_in_names = list(in_names) + list(out_names)
    if partition_name is not None:
        all_in_names.append(partition_name)

    def _body(*args):
        operands = list(args)
        if partition_name is not None:
            operands.append(bass2jax.partition_id_tensor())
        outs = bass2jax._bass_exec_p.bind(
            *operands,
            out_avals=tuple(out_avals),
            in_names=tuple(all_in_names),
            out_names=tuple(out_names),
            lowering_input_output_aliases=(),
            sim_require_finite=True,
            sim_require_nnan=True,
            nc=nc,
        )
        return tuple(outs)

    devices = jax.devices()[:8]
    mesh = Mesh(np.asarray(devices), ("core",))
    n_outs = len(out_avals)
    sharded = jax.jit(
        shard_map(_body, mesh=mesh,
                  in_specs=(PartitionSpec("core"),) * (n_params + n_outs),
                  out_specs=(PartitionSpec("core"),) * n_outs,
                  check_rep=False),
        keep_unused=True)
    csharding = NamedSharding(mesh, PartitionSpec("core"))

    def prepare(maps, device=True):
        import jax as _jax
        per_core = [[np.asarray(m[nm]) for nm in in_names] for m in maps]
        concat_in = [np.concatenate([per_core[c][i] for c in range(8)], axis=0)
                     for i in range(n_params)]
        concat_zeros = [np.zeros((8 * z.shape[0], *z.shape[1:]), z.dtype)
                        for z in zero_outs]
        args = concat_in + concat_zeros
        if device:
            args = [_jax.device_put(a, csharding) for a in args]
            _jax.block_until_ready(args)
        return args

    def call(args):
        return sharded(*args)

    def to_results(out_arrs):
        return [
            {nm: np.asarray(out_arrs[i]).reshape(8, *out_avals[i].shape)[c]
             for i, nm in enumerate(out_names)}
            for c in range(8)
        ]

    def runner(maps):
        return to_results(call(prepare(maps)))

    runner.prepare = prepare
    runner.call = call
    runner.to_results = to_results
    runner.sharding = csharding
    return runner


def _get_runner():
    if "runner" not in _CACHE:
        _CACHE["runner"] = _make_runner(_build_program())
    return _CACHE["runner"]


def _prep_core_inputs(b, r, h, inputs):
    """Host-side shard/fold for core (batch b, branch r, half h)."""
    p = "fwd" if r == 0 else "bwd"
    x = np.asarray(inputs["x"], np.float32)
    ln_g = np.asarray(inputs["ln_g"], np.float32)
    ln_b = np.asarray(inputs["ln_b"], np.float32)
    in_w = np.asarray(inputs[p + "_in_w"], np.float32)
    conv_w = np.asarray(inputs[p + "_conv_w"], np.float32)
    conv_b = np.asarray(inputs[p + "_conv_b"], np.float32)
    xproj_w = np.asarray(inputs[p + "_xproj_w"], np.float32)
    dt_w = np.asarray(inputs[p + "_dt_w"], np.float32)
    dt_b = np.asarray(inputs[p + "_dt_b"], np.float32)
    A_log = np.asarray(inputs[p + "_A_log"], np.float32)
    Dp = np.asarray(inputs[p + "_D"], np.float32)
    out_w = np.asarray(inputs[p + "_out_w"], np.float32)
    proj_w = np.asarray(inputs["proj_w"], np.float32)

    sl = slice(h * DL, (h + 1) * DL)
    xb = x[b]
    if r == 1:
        xb = xb[::-1]
    xT = np.ascontiguousarray(xb.T).astype(BF16)

    W = np.concatenate([in_w[sl], in_w[D_INNER + h * DL:D_INNER + (h + 1) * DL]], 0)
    W = W * ln_g[None, :]                      # [2*DL, D_MODEL], ln_g folded
    cb = W @ ln_b                              # [2*DL]
    cb_x, cb_z = cb[:DL], cb[DL:]
    w_inT = np.ascontiguousarray(W.T).astype(BF16)

    cwl = conv_w[sl]                           # [DL, 4]
    conv_b_eff = conv_b[sl] + cb_x * cwl.sum(1)
    conv_diag = np.zeros((128, NBLK * D_CONV * 128), np.float32)
    for D in range(NBLK):
        for k in range(D_CONV):
            blk = (D * D_CONV + k) * 128
            conv_diag[np.arange(128), blk + np.arange(128)] = \
                cwl[D * 128:(D + 1) * 128, k]
    dp_diag = np.zeros((128, NBLK * 128), np.float32)
    for D in range(NBLK):
        dp_diag[np.arange(128), D * 128 + np.arange(128)] = \
            Dp[sl][D * 128:(D + 1) * 128]

    def col(v):
        return np.ascontiguousarray(v.reshape(NBLK, 128).T).astype(np.float32)

    A = -np.exp(A_log[sl])                     # [DL, 16]
    a_cols = np.ascontiguousarray(
        A.reshape(NBLK, 128, D_STATE).transpose(1, 0, 2).reshape(128, NBLK * D_STATE)
    ).astype(np.float32)

    w_fold = proj_w[:, r * D_MODEL:(r + 1) * D_MODEL] @ out_w[:, sl]  # [dm, DL]

    return {
        "xT": xT,
        "w_inT": w_inT,
        "xproj_wT": np.ascontiguousarray(xproj_w[:, sl].T).astype(BF16),
        "dt_wT": np.ascontiguousarray(dt_w[sl].T).astype(BF16),
        "w_foldT": np.ascontiguousarray(w_fold.T).astype(BF16),
        "conv_diag": conv_diag.astype(BF16),
        "dp_diag": dp_diag.astype(BF16),
        "ident_p": np.eye(128, dtype=np.float32).astype(BF16),
        "conv_b_c": col(conv_b_eff),
        "silu_zb_c": col(cb_z),
        "dt_b_c": col(dt_b[sl]),
        "a_cols": a_cols,
    }


def make_in_maps(inputs):
    maps = []
    for c in range(8):
        b, r, h = c // 4, (c // 2) % 2, c % 2
        maps.append(_prep_core_inputs(b, r, h, inputs))
    return maps


def gather(inputs, results):
    x = np.asarray(inputs["x"], np.float32)
    proj_b = np.asarray(inputs["proj_b"], np.float32)
    out = x + proj_b[None, None, :]
    for c in range(8):
        b, r, h = c // 4, (c // 2) % 2, c % 2
        part = np.asarray(results[c]["y_part"], np.float32)
        if r == 1:
            part = part[::-1]
        out[b] += part
    return out


def kernel(**inputs) -> np.ndarray:
    runner = _get_runner()
    maps = make_in_maps(inputs)
    results = runner(maps)
    return gather(inputs, results)
